# revision 1
# baseline (speedup 1.0000x reference)
"""AtomicOrbitals forward kernel for Trainium2 (Bass/Tile), 8-core SPMD.

Math: for each point p (walker,electron) and basis j:
  ao_bas[p,j] = norm_cst[j]*bas_coeffs[j] * r^(kx+ky+kz) * exp(-alpha r^2)
              * dx^kx * dy^ky * dz^kz          with d = p - coord(j), r=|d|
Tricks:
  * r^(kx+ky+kz)*dx^kx*dy^ky*dz^kz = (r dx)^kx (r dy)^ky (r dz)^kz, so each
    factor is u^k with k in {0,1,2} and u = r*d.
  * u^k = Square(m2*u + m0) + m1*u with one-hot (m2,m1,m0): the quadratic
    evaluation runs on the ACT engine (per-partition scale/bias), leaving one
    fused DVE op per coordinate.
  * r^2 = |c|^2 - 2 c.p + |p|^2 on the TensorEngine as a K=5 matmul against
    host-augmented rows [x, y, z, 1, |p|^2].
  * scatter-add over index_ctr is a matmul with a (256,160) selection matrix
    (coeffs folded in).
  * ACT ops are phase-batched (sqrt / exp+square) per 4-chunk group so
    activation-table reloads amortize.

Layout: basis on partitions (2 halves of 128), points on the free axis.
Data-parallel over walkers: 8 cores x 128 walkers (8192 points each).
"""

import numpy as np

NBATCH = 1024
NELEC = 64
NATOMS = 16
NSH = 16
NBAS = 256
NORB = 160
NCORES = 8
B_LOC = NBATCH // NCORES          # 128 walkers per core
NPTS = B_LOC * NELEC              # 8192 points per core
F = 512                           # points per chunk
NCHUNK = NPTS // F                # 16
G = 4                             # chunks per ACT-batching group

# consts_sb column layout (per half h: columns h*13 + k)
BX, BY, BZ, NEGA = 0, 1, 2, 3
M2X, M1X, M0X = 4, 5, 6
M2Y, M1Y, M0Y = 7, 8, 9
M2Z, M1Z, M0Z = 10, 11, 12
NCST = 13

# Tunable config: n_sq = how many coords (from z backwards) evaluate the
# power polynomial via ACT Square; the rest use A=ts(2x) + B=stt.
# TT-class ops are greedily balanced between DVE and GPSIMD using the
# per-op cost estimates below (ns, F=512).
CFG = {"n_sq": 3, "uvw_decomp": False, "osb_act": True}
# scalar_tensor_tensor ("stt") is NOT a legal Pool opcode; tt/ts are.
import os
# GPSIMD elementwise measured ~5-10x slower than the cost model claims on
# real HW (software Q7 dispatch) -- keep it effectively disabled.
_COST = {
    ("vector", "tt"): 594, ("vector", "ts"): 327, ("vector", "stt"): 594,
    ("gpsimd", "tt"): int(os.environ.get("POOL_TT", "99999")),
    ("gpsimd", "ts"): int(os.environ.get("POOL_TS", "99999")),
}

_PROGRAM_CACHE = {}


class _Balancer:
    def __init__(self, fixed_dve=0.0):
        self.load = {"vector": fixed_dve, "gpsimd": 0.0}

    def pick(self, kind):
        e = min(self.load, key=lambda e: self.load[e] + _COST[(e, kind)])
        self.load[e] += _COST[(e, kind)]
        return e


def build_program(cfg=None, n_iter=1, skip_bcast=False, loop_n=None):
    import concourse.bass as bass
    import concourse.mybir as mybir
    from concourse import bacc, tile
    from contextlib import ExitStack

    f32 = mybir.dt.float32
    Alu = mybir.AluOpType
    Act = mybir.ActivationFunctionType

    cfg = dict(CFG, **(cfg or {}))
    n_sq = cfg["n_sq"]
    # coords 2,1,0 -> square-form for the last n_sq coords
    pform = ["sq" if k >= 3 - n_sq else "ab" for k in range(3)]
    bal = _Balancer(fixed_dve=64 * 291.0)  # osb copies stay on DVE (total ns)

    nc = bacc.Bacc(None, target_bir_lowering=False)

    def eng(kind):
        if kind == "stt":
            bal.load["vector"] += _COST[("vector", "stt")]
            return nc.vector
        return getattr(nc, bal.pick(kind))

    aug = nc.dram_tensor("aug", [5, NPTS], f32, kind="ExternalInput")
    consts = nc.dram_tensor("consts", [128, 2 * NCST], f32, kind="ExternalInput")
    catom = nc.dram_tensor("catom", [5, NBAS], f32, kind="ExternalInput")
    smat = nc.dram_tensor("smat", [128, 2 * NORB], f32, kind="ExternalInput")
    out = nc.dram_tensor("out", [NPTS, NORB], f32, kind="ExternalOutput")

    with tile.TileContext(nc) as tc, ExitStack() as ctx:
        const_pool = ctx.enter_context(tc.tile_pool(name="const", bufs=1))
        consts_sb = const_pool.tile([128, 2 * NCST], f32)
        nc.sync.dma_start(consts_sb[:], consts[:])
        catom_sb = const_pool.tile([5, NBAS], f32)
        nc.sync.dma_start(catom_sb[:], catom[:])
        smat_sb = const_pool.tile([128, 2 * NORB], f32)
        nc.sync.dma_start(smat_sb[:], smat[:])

        bcast = ctx.enter_context(tc.tile_pool(name="bcast", bufs=G + 1))
        augp = ctx.enter_context(tc.tile_pool(name="augp", bufs=G + 1))
        r2_pool = ctx.enter_context(tc.tile_pool(name="r2", bufs=2, space="PSUM"))
        r2s_pool = ctx.enter_context(tc.tile_pool(name="r2s", bufs=G + 1))
        rt_pool = ctx.enter_context(tc.tile_pool(name="rt", bufs=G + 1))
        e_pool = ctx.enter_context(tc.tile_pool(name="ep", bufs=G + 1))
        o_pool = ctx.enter_context(tc.tile_pool(name="o", bufs=2, space="PSUM"))
        wk = ctx.enter_context(tc.tile_pool(name="wk", bufs=2))
        ao_pool = ctx.enter_context(tc.tile_pool(name="ao", bufs=3))

        aug_ap = aug[:]

        def cst(h, k):
            col = h * NCST + k
            return consts_sb[:, col:col + 1]

        from contextlib import nullcontext
        loop_ctx = tc.For_i(0, loop_n, 1) if loop_n else nullcontext()
        with loop_ctx:
          for _it in range(n_iter):
            for g in range(NCHUNK // G):
              chunks = list(range(g * G, (g + 1) * G))
              r2s = {}
              r_t = {}
              e_t = {}
              xyz = {}
              # phase A: DMAs, r^2 matmuls, psum->sbuf copy (ACT, Copy: no
              # table load in any set)
              for c in chunks:
                  sl = slice(c * F, (c + 1) * F)
                  xyz[c] = bcast.tile([128, 3 * F], f32, name="xyz", tag="xyz")
                  if not (skip_bcast and _it > 0):
                      src = bass.AP(tensor=aug_ap.tensor, offset=c * F,
                                    ap=[[0, 128], [NPTS, 3], [1, F]])
                      nc.sync.dma_start(
                          xyz[c][:].rearrange("p (k f) -> p k f", k=3), src)
                  else:
                      nc.gpsimd.memset(xyz[c][:, 0:4], 0.0)
                  aug_t = augp.tile([5, F], f32, name="aug_t", tag="aug_t")
                  nc.scalar.dma_start(aug_t[:], aug_ap[:, sl])
                  r2 = r2_pool.tile([128, 2 * F], f32, name="r2", tag="r2")
                  for h in range(2):
                      nc.tensor.matmul(
                          r2[:, h * F:(h + 1) * F],
                          lhsT=catom_sb[:, h * 128:(h + 1) * 128],
                          rhs=aug_t[:],
                          start=True, stop=True,
                      )
                  r2s[c] = r2s_pool.tile([128, 2 * F], f32, name="r2s", tag="r2s")
                  nc.scalar.copy(r2s[c][:], r2[:])
              # phase B: sqrts (sqrt table)
              for c in chunks:
                  r_t[c] = rt_pool.tile([128, 2 * F], f32, name="r_t", tag="r_t")
                  nc.scalar.activation(r_t[c][:], r2s[c][:], Act.Sqrt)
              # phase C: exps (exp table; Squares later also live in it)
              for c in chunks:
                  e_t[c] = e_pool.tile([128, 2 * F], f32, name="e_t", tag="e_t")
                  for h in range(2):
                      nc.scalar.activation(
                          e_t[c][:, h * F:(h + 1) * F],
                          r2s[c][:, h * F:(h + 1) * F],
                          Act.Exp, scale=cst(h, NEGA))
              # phase D: elementwise chains + contraction
              for c in chunks:
                  ao = []
                  for h in range(2):
                      rth = r_t[c][:, h * F:(h + 1) * F]
                      # u_k = (x_b - b_k) * r
                      uvw = []
                      for k, bk in enumerate((BX, BY, BZ)):
                          t = wk.tile([128, F], f32, name=f"uvw{k}", tag=f"uvw{k}")
                          if cfg["uvw_decomp"]:
                              d = wk.tile([128, F], f32, name=f"d{k}", tag=f"d{k}")
                              eng("ts").tensor_scalar(
                                  d[:], xyz[c][:, k * F:(k + 1) * F],
                                  cst(h, bk), None, op0=Alu.subtract)
                              eng("tt").tensor_tensor(
                                  t[:], d[:], rth, op=Alu.mult)
                          else:
                              eng("stt").scalar_tensor_tensor(
                                  t[:], xyz[c][:, k * F:(k + 1) * F], cst(h, bk),
                                  rth, op0=Alu.subtract, op1=Alu.mult)
                          uvw.append(t)
                      # power select P_k = u^k, two forms:
                      #  sq: P_k = Square(m2*u + m0) [ACT] + m1*u [fused stt]
                      #  ab: B_k = (m2*u + m1)*u, "+ m0" deferred to the chain
                      P = []  # (tile, pending_m0_col or None)
                      for k, (m2, m1, m0) in enumerate((
                              (M2X, M1X, M0X), (M2Y, M1Y, M0Y), (M2Z, M1Z, M0Z))):
                          if pform[k] == "sq":
                              sq = wk.tile([128, F], f32, name=f"sq{k}", tag=f"sq{k}")
                              nc.scalar.activation(
                                  sq[:], uvw[k][:], Act.Square,
                                  bias=cst(h, m0), scale=cst(h, m2))
                              p = wk.tile([128, F], f32, name=f"P{k}", tag=f"P{k}")
                              eng("stt").scalar_tensor_tensor(
                                  p[:], uvw[k][:], cst(h, m1), sq[:],
                                  op0=Alu.mult, op1=Alu.add)
                              P.append((p, None))
                          else:
                              a = wk.tile([128, F], f32, name=f"A{k}", tag=f"A{k}")
                              eng("ts").tensor_scalar(
                                  a[:], uvw[k][:], cst(h, m2), cst(h, m1),
                                  op0=Alu.mult, op1=Alu.add)
                              b = wk.tile([128, F], f32, name=f"B{k}", tag=f"B{k}")
                              eng("tt").tensor_tensor(
                                  b[:], a[:], uvw[k][:], op=Alu.mult)
                              p2 = wk.tile([128, F], f32, name=f"Pf{k}", tag=f"Pf{k}")
                              eng("ts").tensor_scalar(
                                  p2[:], b[:], cst(h, m0), None, op0=Alu.add)
                              P.append((p2, None))
                      # ao_h = Px * Py * Pz * e  (deferred +m0 folded via stt)
                      prev = e_t[c][:, h * F:(h + 1) * F]
                      for j, (pt, m0col) in enumerate(P):
                          last = j == len(P) - 1
                          dst = (ao_pool.tile([128, F], f32, name=f"ao{h}",
                                              tag=f"ao{h}") if last else
                                 wk.tile([128, F], f32, name=f"t{j}", tag=f"t{j}"))
                          assert m0col is None
                          eng("tt").tensor_tensor(
                              dst[:], pt[:], prev, op=Alu.mult)
                          prev = dst[:]
                      ao.append(dst)
                  for m in range(F // 128):
                      ot = o_pool.tile([128, NORB], f32, name="ot", tag="ot")
                      nc.tensor.matmul(
                          ot[:], lhsT=ao[0][:, m * 128:(m + 1) * 128],
                          rhs=smat_sb[:, 0:NORB], start=True, stop=False)
                      nc.tensor.matmul(
                          ot[:], lhsT=ao[1][:, m * 128:(m + 1) * 128],
                          rhs=smat_sb[:, NORB:2 * NORB], start=False, stop=True)
                      osb = ao_pool.tile([128, NORB], f32, name="osb", tag="osb")
                      if cfg["osb_act"]:
                          nc.scalar.copy(osb[:], ot[:])
                      else:
                          nc.vector.tensor_copy(osb[:], ot[:])
                      r0 = c * F + m * 128
                      nc.sync.dma_start(out[r0:r0 + 128, :], osb[:])
    nc.compile()
    return nc


def prep_inputs(pos, atom_coords, bas_exp, bas_coeffs, norm_cst,
                bas_kx, bas_ky, bas_kz, index_ctr):
    """Host-side preprocessing -> per-core in_maps."""
    pos = np.asarray(pos, np.float32)
    atom_coords = np.asarray(atom_coords, np.float32)
    bas_exp = np.asarray(bas_exp, np.float32)
    bas_coeffs = np.asarray(bas_coeffs, np.float32)
    norm_cst = np.asarray(norm_cst, np.float32)
    kx = np.asarray(bas_kx); ky = np.asarray(bas_ky); kz = np.asarray(bas_kz)
    idx = np.asarray(index_ctr)

    bc = np.repeat(atom_coords, NSH, axis=0)          # (256,3)
    cc = (norm_cst * bas_coeffs).astype(np.float32)

    consts = np.zeros((128, 2 * NCST), np.float32)
    catom = np.zeros((5, NBAS), np.float32)
    smat = np.zeros((128, 2 * NORB), np.float32)
    for h in range(2):
        s = slice(h * 128, (h + 1) * 128)
        consts[:, h * NCST + BX] = bc[s, 0]
        consts[:, h * NCST + BY] = bc[s, 1]
        consts[:, h * NCST + BZ] = bc[s, 2]
        consts[:, h * NCST + NEGA] = -bas_exp[s]
        for col, karr in ((M2X, kx), (M2Y, ky), (M2Z, kz)):
            consts[:, h * NCST + col] = (karr[s] == 2)
            consts[:, h * NCST + col + 1] = (karr[s] == 1)
            consts[:, h * NCST + col + 2] = (karr[s] == 0)
        smat[np.arange(128), h * NORB + idx[s]] = cc[s]
    catom[0:3, :] = -2.0 * bc.T
    catom[3, :] = (bc * bc).sum(axis=1)
    catom[4, :] = 1.0

    in_maps = []
    for i in range(NCORES):
        p = pos[i * B_LOC:(i + 1) * B_LOC].reshape(-1, 3)   # (8192,3)
        augi = np.empty((5, NPTS), np.float32)
        augi[0:3] = p.T
        augi[3] = 1.0
        augi[4] = (p * p).sum(axis=1)
        in_maps.append({"aug": augi, "consts": consts, "catom": catom,
                        "smat": smat})
    return in_maps


def kernel(pos, atom_coords, bas_exp, bas_coeffs, norm_cst,
           bas_kx, bas_ky, bas_kz, index_ctr, norb, **_unused):
    from concourse.bass_utils import run_bass_kernel_spmd

    if "nc" not in _PROGRAM_CACHE:
        _PROGRAM_CACHE["nc"] = build_program()
    nc = _PROGRAM_CACHE["nc"]

    in_maps = prep_inputs(pos, atom_coords, bas_exp, bas_coeffs, norm_cst,
                          bas_kx, bas_ky, bas_kz, index_ctr)
    res = run_bass_kernel_spmd(nc, in_maps, list(range(NCORES)))
    outs = [np.asarray(res.results[i]["out"]).reshape(B_LOC, NELEC, NORB)
            for i in range(NCORES)]
    return np.concatenate(outs, axis=0)



# revision 2
# speedup vs baseline: 2.7894x; 2.7894x over previous
"""AtomicOrbitals forward kernel for Trainium2 (Bass/Tile), 8-core SPMD.

Log-domain formulation: for point p, basis j (atom a=j//16, d = p - c_a):
  ao_bas[p,j] = cc_j * r^n * exp(-alpha r^2) * dx^kx dy^ky dz^kz
              = cc_j * sigma * exp(z)
  z     = -alpha*r^2 + (n/4)ln((r^2)^2) + sum_c (k_c/2) ln(d_c^2)
  sigma = (-1)^{#(c: k_c odd, d_c<0)}

Per-atom planes (16 atoms, not 256 bases) feed selection matmuls; one
input DMA per chunk loads X[0:64] from a host-packed plane stack:
  X [80,F]: rows 0:5 aug=[x,y,z,1,|p|^2], rows 5+16c+a coord-c bcast,
            rows 53:64 zero pad, rows 64:80 r^2 (copied from K=5 matmul
            out vs X[0:5]).
  D = X - b ; D2 = D^2 (ACT Square) ; stackln = ln(D2 + 1e-30) (one ACT op;
            r^2 rows become 2 ln r^2 weighted n/4; pad rows ln(eps), wt 0)
  z-MM:  Wz.T @ stackln + Wzr2.T @ X[64:80]   (f32r, alpha folded in)
  s-MM:  Ws.T @ (X < b)  -> s'' in {0,2,4,6};  sigma' = (s''&2)-1 = -sigma
  ao = exp(z) * sigma'  (bf16) ; contraction smat(-cc).T @ ao, orbital-major
  out [80, 2*NPTS] bf16 (orb half-blocks side by side), one DMA per chunk;
  host reassembles/transposes.

Data-parallel over walkers: 8 cores x 128 walkers (8192 points each).
"""

import numpy as np
import ml_dtypes

NBATCH = 1024
NELEC = 64
NATOMS = 16
NSH = 16
NBAS = 256
NORB = 160
NCORES = 8
B_LOC = NBATCH // NCORES          # 128 walkers per core
NPTS = B_LOC * NELEC              # 8192 points per core
F = 512                           # points per chunk
NCHUNK = NPTS // F                # 16

LNB = 1e-30                       # ln(x + LNB): kills ln(0) -> -inf*0 NaNs
CROW = [5, 21, 37]                # X row of coord-c block (atom a at +a)
R2ROW = 64


def round_f32r(a):
    """Round fp32 array to the PE's f32r format: RNE to 11 mantissa bits."""
    b = np.ascontiguousarray(a, np.float32).view(np.uint32)
    b = (b + 0x7FF + ((b >> 12) & 1)) & np.uint32(0xFFFFF000)
    return b.view(np.float32)


CFG = {
    "sign_mode": "xor",    # "xor": flip bf16 sign bit by parity | "int"
    "r2copy_act": True,    # r2 PSUM->X copy on ACT (else DVE)
    "G": 2,                # chunks per ACT-table phase group (PAIR-multiple)
}

_PROGRAM_CACHE = {}


def build_program(cfg=None, n_iter=1, loop_n=None):
    import concourse.bass as bass
    import concourse.mybir as mybir
    from concourse import bacc, tile
    from contextlib import ExitStack, nullcontext

    f32 = mybir.dt.float32
    bf16 = mybir.dt.bfloat16
    f32r = mybir.dt.float32r
    i32 = mybir.dt.int32
    i16 = mybir.dt.int16
    Alu = mybir.AluOpType
    Act = mybir.ActivationFunctionType

    cfg = dict(CFG, **(cfg or {}))
    G = cfg["G"]

    nc = bacc.Bacc(None, target_bir_lowering=False)

    f16 = mybir.dt.float16
    xyzb = nc.dram_tensor("xyzb", [96, NPTS], f16, kind="ExternalInput")
    wz = nc.dram_tensor("wz", [112, 2 * 128], f16, kind="ExternalInput")
    ws = nc.dram_tensor("ws", [96, 2 * 128], bf16, kind="ExternalInput")
    smat = nc.dram_tensor("smat", [128, 4 * 80], bf16, kind="ExternalInput")
    catom5 = nc.dram_tensor("catom5", [5, 16], f16, kind="ExternalInput")
    bvec = nc.dram_tensor("bvec", [96, 1], f32, kind="ExternalInput")
    lnb = nc.dram_tensor("lnb", [128, 1], f32, kind="ExternalInput")
    out = nc.dram_tensor("out", [80, 2 * NPTS], bf16, kind="ExternalOutput")

    with tile.TileContext(nc) as tc, ExitStack() as ctx:
        cp = ctx.enter_context(tc.tile_pool(name="const", bufs=1))
        wz_sb = cp.tile([112, 2 * 128], f16)
        nc.sync.dma_start(wz_sb[:], wz[:])
        ws_sb = cp.tile([96, 2 * 128], bf16)
        nc.sync.dma_start(ws_sb[:], ws[:])
        smat_sb = cp.tile([128, 4 * 80], bf16)
        nc.sync.dma_start(smat_sb[:], smat[:])
        catom_sb = cp.tile([5, 16], f16)
        nc.sync.dma_start(catom_sb[:], catom5[:])
        bvec_sb = cp.tile([96, 1], f32)
        nc.sync.dma_start(bvec_sb[:], bvec[:])
        lnb_sb = cp.tile([128, 1], f32)
        nc.sync.dma_start(lnb_sb[:], lnb[:])

        # Pin the one table set containing Exp+Ln+Square+Copy+Sign so the
        # fixpoint pass never inserts per-phase reloads (scheduler-order
        # independent).
        nc.scalar.add_instruction(mybir.InstLoadActFuncSet(
            name=nc.get_next_instruction_name(), act_func_set_id=6,
            ins=[], outs=[]))

        xp = ctx.enter_context(tc.tile_pool(name="xp", bufs=G // 2 + 2))
        dp = ctx.enter_context(tc.tile_pool(name="dp", bufs=2))
        negp = ctx.enter_context(tc.tile_pool(name="negp", bufs=G // 2 + 2))
        stackp = ctx.enter_context(tc.tile_pool(name="stk", bufs=G // 2 + 2))
        r2pp = ctx.enter_context(tc.tile_pool(name="r2p", bufs=1, space="PSUM"))
        zpp = ctx.enter_context(tc.tile_pool(name="zp", bufs=2, space="PSUM"))
        spp = ctx.enter_context(tc.tile_pool(name="sp", bufs=2, space="PSUM"))
        mp = ctx.enter_context(tc.tile_pool(name="mp", bufs=2))
        opp = ctx.enter_context(tc.tile_pool(name="op", bufs=1, space="PSUM"))
        osbp = ctx.enter_context(tc.tile_pool(name="osb", bufs=2))

        xyzb_ap = xyzb[:]
        out_ap = out[:]

        loop_ctx = tc.For_i(0, loop_n, 1) if loop_n else nullcontext()
        with loop_ctx:
          for _it in range(n_iter):
            for g in range(NCHUNK // G):
              chunks = list(range(g * G, (g + 1) * G))
              Xs, NEGs, stacks = {}, {}, {}
              # --- phase A (ln table set): per-atom planes, 2 chunks/tile ---
              for c0 in chunks[::2]:
                X = xp.tile([96, 2 * F], f16, name="X", tag="X")
                nc.sync.dma_start(X[:], xyzb_ap[:, c0 * F:(c0 + 2) * F])

                r2ps = r2pp.tile([16, 2 * F], f32, name="r2", tag="r2")
                for q in range(2):
                    nc.tensor.matmul(r2ps[:, q * F:(q + 1) * F],
                                     lhsT=catom_sb[:],
                                     rhs=X[0:5, q * F:(q + 1) * F],
                                     start=True, stop=True)
                if cfg["r2copy_act"]:
                    nc.scalar.copy(X[R2ROW:R2ROW + 16, :], r2ps[:])
                else:
                    nc.vector.tensor_copy(X[R2ROW:R2ROW + 16, :], r2ps[:])

                D = dp.tile([96, 2 * F], f32, name="D", tag="D")
                nc.vector.tensor_scalar(D[:], X[:], bvec_sb[:], None,
                                        op0=Alu.subtract)
                NEG = negp.tile([96, 2 * F], bf16, name="NEG", tag="NEG")
                nc.vector.tensor_scalar(NEG[:], X[:], bvec_sb[:],
                                        None, op0=Alu.is_lt)
                D2 = dp.tile([96, 2 * F], f32, name="D2", tag="D2")
                nc.scalar.activation(D2[:], D[:], Act.Square)
                stackln = stackp.tile([112, 2 * F], f16, name="stackln",
                                      tag="stk")
                nc.scalar.activation(stackln[0:96, :], D2[:], Act.Ln,
                                     bias=lnb_sb[0:96, :])
                nc.vector.tensor_copy(stackln[96:112, :], X[0:16, :])
                Xs[c0], NEGs[c0], stacks[c0] = X, NEG, stackln

              # --- phase B (exp table set): per-basis + contraction ---
              for c in chunks:
                c0, q = c - (c % 2), c % 2
                X, NEG, stackln = Xs[c0], NEGs[c0], stacks[c0]
                qs = slice(q * F, (q + 1) * F)
                mag = mp.tile([128, 2 * F], bf16, name="mag", tag="mag")
                s_i = mp.tile([128, 2 * F], i16, name="s_i", tag="s_i")
                for h in range(2):
                    zh = zpp.tile([128, F], f32, name="z", tag="z")
                    nc.tensor.matmul(
                        zh[:], lhsT=wz_sb[:, h * 128:(h + 1) * 128],
                        rhs=stackln[:, qs], start=True, stop=True)
                    sh = spp.tile([128, F], f32, name="s", tag="s")
                    nc.tensor.matmul(
                        sh[:], lhsT=ws_sb[:, h * 128:(h + 1) * 128],
                        rhs=NEG[:, qs], start=True, stop=True)
                    nc.scalar.activation(mag[:, h * F:(h + 1) * F], zh[:],
                                         Act.Exp)
                    nc.vector.tensor_copy(s_i[:, h * F:(h + 1) * F], sh[:])

                ao = mp.tile([128, 2 * F], bf16, name="ao", tag="ao")
                if cfg["sign_mode"] == "xor":
                    # (s<<14) wraps mod 2^16 to parity<<15; xor flips mag's
                    # bf16 sign bit.
                    SM = mp.tile([128, 2 * F], i16, name="SM", tag="SM")
                    nc.vector.tensor_scalar(SM[:], s_i[:], 14, None,
                                            op0=Alu.logical_shift_left)
                    nc.vector.tensor_tensor(ao[:].bitcast(i16),
                                            mag[:].bitcast(i16), SM[:],
                                            op=Alu.bitwise_xor)
                else:
                    s_a = mp.tile([128, 2 * F], i16, name="s_a", tag="s_a")
                    nc.vector.tensor_scalar(s_a[:], s_i[:], 2, None,
                                            op0=Alu.bitwise_and)
                    sigma = mp.tile([128, 2 * F], bf16, name="sig", tag="sig")
                    nc.vector.tensor_scalar(sigma[:], s_a[:], 1, None,
                                            op0=Alu.subtract)
                    nc.vector.tensor_tensor(ao[:], mag[:], sigma[:],
                                            op=Alu.mult)

                osb = osbp.tile([80, 2 * F], bf16, name="osb", tag="osb")
                for oh in range(2):
                    obh = opp.tile([80, F], f32, name=f"obh{oh}", tag=f"obh{oh}")
                    for h in range(2):
                        nc.tensor.matmul(
                            obh[:],
                            lhsT=smat_sb[:, (2 * h + oh) * 80:(2 * h + oh + 1) * 80],
                            rhs=ao[:, h * F:(h + 1) * F],
                            start=(h == 0), stop=(h == 1))
                    if oh == 0:
                        nc.scalar.copy(osb[:, 0:F], obh[:])
                    else:
                        nc.vector.tensor_copy(osb[:, F:2 * F], obh[:])
                dst = bass.AP(tensor=out_ap.tensor, offset=c * F,
                              ap=[[2 * NPTS, 80], [NPTS, 2], [1, F]])
                nc.scalar.dma_start(
                    dst, osb[:].rearrange("p (t f) -> p t f", t=2))
    nc.compile()
    return nc


def prep_inputs(pos, atom_coords, bas_exp, bas_coeffs, norm_cst,
                bas_kx, bas_ky, bas_kz, index_ctr):
    """Host-side preprocessing -> per-core in_maps."""
    pos = np.asarray(pos, np.float32)
    atom_coords = np.asarray(atom_coords, np.float32)
    bas_exp = np.asarray(bas_exp, np.float32)
    bas_coeffs = np.asarray(bas_coeffs, np.float32)
    norm_cst = np.asarray(norm_cst, np.float32)
    kx = np.asarray(bas_kx).astype(np.float32)
    ky = np.asarray(bas_ky).astype(np.float32)
    kz = np.asarray(bas_kz).astype(np.float32)
    idx = np.asarray(index_ctr)

    cc = (norm_cst * bas_coeffs).astype(np.float32)
    n_j = kx + ky + kz
    ks = [kx, ky, kz]

    wz = np.zeros((112, 256), np.float32)
    ws = np.zeros((96, 256), np.float32)
    smat = np.zeros((128, 4 * 80), np.float32)
    for h in range(2):
        for j in range(128):
            J = h * 128 + j
            a = J // NSH
            al = bas_exp[J]
            col = h * 128 + j
            wz[R2ROW + a, col] = n_j[J] / 4.0
            # -alpha*r^2 via raw aug rows (96:101 = [x,y,z,1,|p|^2]):
            wz[96 + 0, col] = 2.0 * al * atom_coords[a, 0]
            wz[96 + 1, col] = 2.0 * al * atom_coords[a, 1]
            wz[96 + 2, col] = 2.0 * al * atom_coords[a, 2]
            wz[96 + 3, col] = -al * (atom_coords[a] * atom_coords[a]).sum()
            wz[96 + 4, col] = -al
            for ci in range(3):
                wz[CROW[ci] + a, col] = ks[ci][J] / 2.0
                ws[CROW[ci] + a, col] = 2.0 * (ks[ci][J] % 2)
            oh = idx[J] // 80
            sgn = 1.0 if CFG["sign_mode"] == "xor" else -1.0
            smat[j, (2 * h + oh) * 80 + (idx[J] - oh * 80)] += sgn * cc[J]

    catom5 = np.zeros((5, 16), np.float32)
    catom5[0:3, :] = -2.0 * atom_coords.T
    catom5[3, :] = (atom_coords * atom_coords).sum(axis=1)
    catom5[4, :] = 1.0
    bvec = np.zeros((96, 1), np.float32)
    for ci in range(3):
        bvec[CROW[ci]:CROW[ci] + 16, 0] = atom_coords[:, ci]
    lnb = np.full((128, 1), LNB, np.float32)

    wz = wz.astype(np.float16)
    catom5 = catom5.astype(np.float16)
    ws_b = ws.astype(ml_dtypes.bfloat16)
    smat_b = smat.astype(ml_dtypes.bfloat16)

    in_maps = []
    for i in range(NCORES):
        p = pos[i * B_LOC:(i + 1) * B_LOC].reshape(-1, 3)   # (8192,3)
        xyzb = np.zeros((96, NPTS), np.float32)
        xyzb[0:3] = p.T
        xyzb[3] = 1.0
        xyzb[4] = (p * p).sum(axis=1)
        for ci in range(3):
            xyzb[CROW[ci]:CROW[ci] + 16] = p.T[ci]
        xyzb = xyzb.astype(np.float16)
        in_maps.append({"xyzb": xyzb, "wz": wz, "ws": ws_b,
                        "smat": smat_b, "catom5": catom5, "bvec": bvec,
                        "lnb": lnb})
    return in_maps


def kernel(pos, atom_coords, bas_exp, bas_coeffs, norm_cst,
           bas_kx, bas_ky, bas_kz, index_ctr, norb, **_unused):
    from concourse.bass_utils import run_bass_kernel_spmd

    if "nc" not in _PROGRAM_CACHE:
        _PROGRAM_CACHE["nc"] = build_program()
    nc = _PROGRAM_CACHE["nc"]

    in_maps = prep_inputs(pos, atom_coords, bas_exp, bas_coeffs, norm_cst,
                          bas_kx, bas_ky, bas_kz, index_ctr)
    res = run_bass_kernel_spmd(nc, in_maps, list(range(NCORES)))
    outs = []
    for i in range(NCORES):
        o2 = np.asarray(res.results[i]["out"]).astype(np.float32)
        full = np.concatenate([o2[:, :NPTS], o2[:, NPTS:]], axis=0)
        outs.append(full.T.reshape(B_LOC, NELEC, NORB))
    return np.concatenate(outs, axis=0)
